# revision 1
# baseline (speedup 1.0000x reference)
"""BlockwiseDense Trainium2 kernel (8 NeuronCores, sharded over out_blocks).

Math (per reference):
    w = rram_quantize(relu(cores))          # snap to 256 log-spaced levels
    y[b,i,j,k] = sum_l w[i,j,k,l] * x[b,j,l]

Level index n = sat_u8(rne(s(w))), s = MULT*ln((A-w)/B) + C0, computed
via two engine-balanced paths per granule of j-blocks:
  "ln"  : t = Ln((A-w)/B)  (ACT)  ; n = sat_u8(t*MULT + C0)  (GpSimd ts)
  "poly": s ~= w*(A2M*w + P1) + P0   [quadratic; 0.06% level flips]
          gg = ts(...) ; hh = tt(w*gg) ; n = ts(hh+P0 -> u8)   (DVE)
then e = Exp(n*ln_r) fp32 (ACT); q = bf16(A - B*e) (GpSimd/DVE ts);
bf16 matmuls (fp16 runs the PE at HALF rate — measured 620ns vs 275ns
for a 512-wide moving operand) accumulate in fp32 PSUM over the two
128-row halves of l; one CAST per granule evicts to bf16.

dtypes: weights fp16 in HBM (bf16 would corrupt the quantize decision),
x and q bf16 (small/relative values - 0.1% each), y bf16 out.

Granules are small at the head (fast pipeline fill) and tail (short
drain), 2-j in the middle; weight DMAs alternate between the sync and
gpsimd rings; engine streams are software-pipelined with the n-chain
one granule ahead of Exp.  Core c takes out_blocks {2c, 2c+1}.
"""

import numpy as np
import ml_dtypes

import concourse.bacc as bacc
import concourse.mybir as mybir
from concourse.tile import TileContext
from concourse.bass_utils import run_bass_kernel_spmd

BATCH = 128
IN_BLOCKS = 16
OUT_BLOCKS = 16
NB = 256
N_CORES = 8
I_PER_CORE = OUT_BLOCKS // N_CORES  # 2
IK = I_PER_CORE * NB  # 512

TAU, G_INF, G_MIN, L = 0.75, 2.0, 0.001, 256
B_SCALE = (G_INF - G_MIN) / (1.0 - float(np.exp(-TAU)))
A_OFF = G_MIN + B_SCALE
MULT = -(L - 1) / TAU
LN_R = -TAU / (L - 1)
_r = float(np.exp(LN_R))
DELTA = float(np.log((1 + _r) / 2) / LN_R)
C0 = 0.5 - DELTA

# poly-n: s(w) = w*(A2M*w + P1) + P0
_c1 = -C0 / 340.0
_g2 = -340.0 - 170.0 * _c1
C1W = -(G_MIN + B_SCALE * _c1)
A2M = 170.0 / (B_SCALE * B_SCALE)
A2B = -(170.0 * G_MIN / B_SCALE + _g2) / B_SCALE
P1 = A2B + C1W * A2M
P0 = C1W * A2B

F32 = mybir.dt.float32
F16 = mybir.dt.float16
BF16 = mybir.dt.bfloat16
U8 = mybir.dt.uint8

# granules: (j-list, path); js contiguous ascending
GSPEC = [
    ([0], "ln"),
    ([1], "ln"),
    ([2, 3], "ln"),
    ([4, 5], "poly"),
    ([6, 7], "poly"),
    ([8, 9], "poly"),
    ([10, 11], "poly"),
    ([12, 13], "ln"),
    ([14], "ln"),
    ([15], "ln"),
]
# q-affine engine per granule ("g" GpSimd fp32->bf16, "v" DVE)
QENG = {8: "v", 9: "v"}
# ring (issuing engine) per granule's weight DMA
WRING = {0: "s", 1: "s", 2: "g", 3: "s", 4: "g", 5: "s", 6: "g", 7: "s", 8: "g", 9: "s"}

_CACHE = {}


class _ForceCombinedLnExpTables:
    """Resolve Ln and Exp to the single table set containing both, so the
    ACT never reloads tables mid-kernel."""

    def __enter__(self):
        self._orig = bacc.get_activation_tables
        Ln = mybir.ActivationFunctionType.Ln
        Exp = mybir.ActivationFunctionType.Exp

        def patched(arch):
            tabs = self._orig(arch)
            out = {}
            for name, fns in tabs.items():
                if name != "natural_log_exp_and_others" and (Ln in fns or Exp in fns):
                    fns = fns - {Ln, Exp}
                out[name] = fns
            return out

        bacc.get_activation_tables = patched
        return self

    def __exit__(self, *exc):
        bacc.get_activation_tables = self._orig


def _build():
    nc = bacc.Bacc(trn_type="TRN2")
    P = 128
    NG = len(GSPEC)

    xt_d = nc.dram_tensor("xt", [P, IN_BLOCKS, 2, BATCH], F16, kind="ExternalInput")
    wt_d = nc.dram_tensor("wt", [P, IN_BLOCKS, 2, IK], F16, kind="ExternalInput")
    y_d = nc.dram_tensor("y", [BATCH, IN_BLOCKS, IK], F16, kind="ExternalOutput")

    flat = "p a b k -> p (a b k)"

    with TileContext(nc) as tc:
        with (
            tc.tile_pool(name="singles", bufs=1) as singles,
            tc.tile_pool(name="wraw", bufs=5) as wpool,
            tc.tile_pool(name="tmid", bufs=5) as tpool,
            tc.tile_pool(name="nidx", bufs=4) as npool,
            tc.tile_pool(name="texp", bufs=4) as epool,
            tc.tile_pool(name="qw", bufs=4) as qpool,
            tc.tile_pool(name="yout", bufs=3) as ypool,
            tc.tile_pool(name="yps", bufs=6, space="PSUM") as yps,
            tc.tile_pool(name="sps", bufs=1, space="PSUM") as sps,
        ):
            wt_t = [None] * NG
            t_t = [None] * NG
            n_t = [None] * NG
            e_t = [None] * NG
            sa_t = [None] * NG
            y_t = [None] * NG
            p_t = [None] * IN_BLOCKS

            def dma_w(g):
                js, _ = GSPEC[g]
                nj = len(js)
                wt_t[g] = wpool.tile([P, nj, 2, IK], F16, name="wraw", tag="wraw")
                e = nc.sync if WRING[g] == "s" else nc.gpsimd
                e.dma_start(out=wt_t[g][:], in_=wt_d[:, js[0] : js[0] + nj])

            def nstage1(g):
                js, path = GSPEC[g]
                fd = len(js) * 2 * IK
                if path == "ln":
                    t_t[g] = tpool.tile([P, fd], F32, name="tln", tag="tmid")
                    nc.scalar.activation(
                        t_t[g][:],
                        wt_t[g][:].rearrange(flat),
                        mybir.ActivationFunctionType.Ln,
                        bias=bias_ln[:, 0:1],
                        scale=-1.0 / B_SCALE,
                    )
                else:
                    t_t[g] = tpool.tile([P, fd], F16, name="tgg", tag="tmid")
                    nc.vector.tensor_scalar(
                        t_t[g][:],
                        wt_t[g][:].rearrange(flat),
                        A2M,
                        P1,
                        mybir.AluOpType.mult,
                        mybir.AluOpType.add,
                    )

            def nstage2(g):
                js, path = GSPEC[g]
                fd = len(js) * 2 * IK
                n_t[g] = npool.tile([P, fd], U8, name="nidx", tag="nidx")
                if path == "ln":
                    nc.gpsimd.tensor_scalar(
                        n_t[g][:],
                        t_t[g][:],
                        MULT,
                        C0,
                        mybir.AluOpType.mult,
                        mybir.AluOpType.add,
                    )
                else:
                    hh = tpool.tile([P, fd], F16, name="thh", tag="tmid")
                    nc.vector.tensor_tensor(
                        hh[:],
                        wt_t[g][:].rearrange(flat),
                        t_t[g][:],
                        mybir.AluOpType.mult,
                    )
                    nc.vector.tensor_scalar(
                        n_t[g][:], hh[:], P0, None, mybir.AluOpType.add
                    )

            def exp_stage(g):
                js, _ = GSPEC[g]
                nj = len(js)
                e_t[g] = epool.tile([P, nj, 2, IK], F16, name="texp", tag="texp")
                nc.scalar.activation(
                    e_t[g][:].rearrange(flat),
                    n_t[g][:],
                    mybir.ActivationFunctionType.Exp,
                    bias=0.0,
                    scale=LN_R,
                )

            def mm_stage(g):
                js, _ = GSPEC[g]
                for jrel, j in enumerate(js):
                    p_t[j] = yps.tile([P, IK], F32, name="yp", tag="yp")
                    for h in range(2):
                        nc.tensor.matmul(
                            s2_ps[:, j : j + 1],
                            xt_sb[:, j, h, :],
                            ones_sb[:],
                            start=(h == 0),
                            stop=(h == 1),
                        )
                        nc.tensor.matmul(
                            p_t[j][:],
                            xt_sb[:, j, h, :],
                            e_t[g][:, jrel, h, :],
                            start=(h == 0),
                            stop=(h == 1),
                        )

            def sa_stage(g):
                js, _ = GSPEC[g]
                nj = len(js)
                sa_t[g] = qpool.tile([P, nj], F32, name="sa", tag="sa")
                nc.vector.tensor_scalar(
                    sa_t[g][:],
                    s2_ps[:, js[0] : js[0] + nj],
                    -A_OFF / B_SCALE,
                    None,
                    mybir.AluOpType.mult,
                )

            def evict_stage(g):
                js, _ = GSPEC[g]
                nj = len(js)
                y_t[g] = ypool.tile([P, nj, IK], F16, name="ysb", tag="ysb")
                for jrel, j in enumerate(js):
                    nc.vector.tensor_scalar(
                        y_t[g][:, jrel, :],
                        p_t[j][:],
                        sa_t[g][:, jrel : jrel + 1],
                        -B_SCALE,
                        mybir.AluOpType.add,
                        mybir.AluOpType.mult,
                    )

            def store_stage(g):
                js, _ = GSPEC[g]
                nc.sync.dma_start(
                    out=y_d[:, js[0] : js[0] + len(js)], in_=y_t[g][:]
                )

            # --- prologue ---
            bias_ln = singles.tile([P, 1], F32)
            nc.vector.memset(bias_ln[:], A_OFF / B_SCALE)
            # dummy tiny activation: forces the Ln/Exp table load to run
            # before the input DMAs hog the rings
            warm = singles.tile([P, 1], F32)
            nc.scalar.activation(
                warm[:], bias_ln[:], mybir.ActivationFunctionType.Exp,
                bias=0.0, scale=0.0,
            )
            ones_sb = singles.tile([P, 1], F16)
            nc.vector.memset(ones_sb[:], 1.0)
            s2_ps = sps.tile([P, IN_BLOCKS], F32)
            # PE warm-up: ~4.5us of back-to-back dummy matmuls raises the
            # HAM clock gate to 2.4 GHz before the real matmuls arrive
            warm_l = singles.tile([P, 16], F16)
            nc.vector.memset(warm_l[:], 0.5)
            warm_r = singles.tile([P, IK], F16)
            nc.vector.memset(warm_r[:], 0.5)
            wm_ps = sps.tile([16, IK], F32)
            for _ in range(8):
                nc.tensor.matmul(
                    wm_ps[:], warm_l[:], warm_r[:], start=True, stop=True
                )
            dma_w(0)  # sync ring
            dma_w(1)  # sync ring
            xt_sb = singles.tile([P, IN_BLOCKS, 2, BATCH], F16)
            nc.gpsimd.dma_start(out=xt_sb[:], in_=xt_d[:])
            dma_w(2)  # gpsimd ring, behind xt
            dma_w(3)  # sync ring

            # --- pipelined main loop ---
            nstage1(0)
            nstage1(1)
            nstage2(0)
            for g in range(NG):
                exp_stage(g)
                if g + 4 < NG:
                    dma_w(g + 4)
                if g + 2 < NG:
                    nstage1(g + 2)
                if g + 1 < NG:
                    nstage2(g + 1)
                mm_stage(g)
                if g >= 1:
                    sa_stage(g - 1)
                    evict_stage(g - 1)
                    store_stage(g - 1)
            sa_stage(NG - 1)
            evict_stage(NG - 1)
            store_stage(NG - 1)

    with _ForceCombinedLnExpTables():
        nc.compile()
    return nc


def _get_nc():
    if "nc" not in _CACHE:
        _CACHE["nc"] = _build()
    return _CACHE["nc"]


def kernel(x: np.ndarray, cores: np.ndarray, _trace=False, _trace_kwargs=None):
    x = np.asarray(x, dtype=np.float32)
    cores = np.asarray(cores, dtype=np.float32)

    xt = np.ascontiguousarray(
        x.T.reshape(IN_BLOCKS, 2, 128, BATCH)
        .transpose(2, 0, 1, 3)
        .astype(np.float16)
    )
    wt_full = (
        cores.reshape(OUT_BLOCKS, IN_BLOCKS, NB, 2, 128)  # i, j, k, h, p
        .transpose(4, 1, 3, 0, 2)  # p, j, h, i, k
        .astype(np.float16)
    )

    in_maps = []
    for c in range(N_CORES):
        wc = np.ascontiguousarray(
            wt_full[:, :, :, c * I_PER_CORE : (c + 1) * I_PER_CORE, :]
        ).reshape(128, IN_BLOCKS, 2, IK)
        in_maps.append({"xt": xt, "wt": wc})

    nc = _get_nc()
    kw = {}
    if _trace:
        kw = dict(trace=True, **(_trace_kwargs or {}))
    out = run_bass_kernel_spmd(nc, in_maps, core_ids=list(range(N_CORES)), **kw)
    if _trace:
        _CACHE["last_result"] = out
    y = np.concatenate(
        [
            r["y"]  # (b, j, (i,k))
            .astype(np.float32)
            .reshape(BATCH, IN_BLOCKS, I_PER_CORE, NB)
            .transpose(0, 2, 1, 3)
            for r in out.results
        ],
        axis=1,
    )
    return y

